# revision 38
# baseline (speedup 1.0000x reference)
"""CBAM attention module (channel gate + spatial softmax attention) on 8 TRN2
NeuronCores, data-parallel over the batch dimension.

Reference computation (per sample b):
    m  = mean_n x[c, n];  mx = max_n x[c, n]
    gate = sigmoid(w2 @ (relu(w1 @ m) + relu(w1 @ mx)))          # (C,)
    x1 = gate[:, None] * x
    s  = sw0 * max_c x1 + sw1 * mean_c x1                        # (N,)
    s  = relu(A * s + Bconst)        # BatchNorm1d(1) eval, folded on host
    att = softmax_n(s)
    out = att[None, :] * x1

Kernel structure per core (2 samples each), HBM-traffic-minimized:
    pass 1: stream x (f32), per-channel sum over n (ScalarE activation
            accum, which also downcasts the tile to f16) and max over n
            (VectorE reduce on the f16 copy); the f16 copy is written back
            to DRAM.  Tiny MLP on TensorE -> gate.
    pass 2: stream x16 (f16); gate-scale split ScalarE(5 chunks)/
            VectorE(3 chunks, 4x tensor_scalar), running-max fold on
            VectorE (f16 TT, 2x); TensorE transposes the 16 blocks per
            tile into one packed f16 PSUM tile, one VectorE reduce
            finishes max-over-c; TensorE matvec (gate stationary, f16)
            accumulates the c-sum in PSUM row-pieces.  Softmax over n in
            the transposed layout (no max-subtract: s>=0 and bounded);
            att replicated across partitions on TensorE.
    pass 3: stream x16 again; out = (x16 * gate) * att on VectorE
            (scalar_tensor_tensor), write back f32.

HBM bytes per core: 134R(f32) + 67W(f16) + 67R + 67R + 134W(f32 out)
= 469 MB vs 536 MB for the all-f32 3R1W scheme.  DMA-bound at
~358 GB/s -> ~1.31 ms floor; measured 1.33-1.41 ms (device-state
noise ~5%).  f16 staging costs ~2.6e-4 relative error, far inside the
2e-2 gate.  The cross-sample emission pipeline (P1(b1)+P2(b0), then
P3(b0)+P2(b1)) keeps every phase DMA-bound; pass-3 emits its eight
x16 loads before the att-piece DMA so the softmax semaphore wait
cannot head-of-line-block the sync sequencer's load stream.
"""

import numpy as np

B, C, N, RATIO = 16, 1024, 16384, 8
H = C // RATIO  # 128
BN_EPS = 1e-5
N_CORES = 8
BC = B // N_CORES  # samples per core

_cached = {}


def _build_nc(NT=4096, BC=BC, C=C, N=N, H=H):
    import concourse.bacc as bacc
    import concourse.mybir as mybir
    import concourse.tile as tile
    from concourse import masks
    from contextlib import ExitStack

    f32 = mybir.dt.float32
    f16 = mybir.dt.float16
    AF = mybir.ActivationFunctionType
    X = mybir.AxisListType.X
    ALU = mybir.AluOpType

    K = C // 128          # c-chunks
    NJ = N // NT          # n-tiles per sample (passes 1/3)
    NB = N // 128         # 128-blocks per sample (transpose-layout columns)
    NT2 = min(2048, NT)   # pass-2 tile (PSUM banks cap the matvec pieces)
    NJ2 = N // NT2
    BPT2 = NT2 // 128     # 128-blocks per pass-2 tile
    MV2 = max(1, NT2 // 512)  # matvec row-pieces per pass-2 tile
    MVW = min(NT2, 512)
    assert NB <= 128

    nc = bacc.Bacc("TRN2", target_bir_lowering=False, debug=False,
                   num_devices=N_CORES)

    x = nc.dram_tensor("x", (BC, C, N), f32, kind="ExternalInput").ap()
    w1t = nc.dram_tensor("w1t", (C, H), f32, kind="ExternalInput").ap()
    w2t = nc.dram_tensor("w2t", (H, C), f32, kind="ExternalInput").ap()
    # params = [sw0, sw1/C, A, Bconst]
    params = nc.dram_tensor("params", (1, 4), f32, kind="ExternalInput").ap()
    out = nc.dram_tensor("out", (BC, C, N), f32, kind="ExternalOutput").ap()

    # f16 staging copy of x, written in pass 1, read in passes 2 and 3.
    # One tensor per sample so pass-2 reads of sample b only wait on the
    # pass-1 writes of the same sample.
    x16s = [nc.dram_tensor(f"x16_{b}", (C, N), f16, kind="Internal").ap()
            for b in range(BC)]
    att_dram = nc.dram_tensor("att_scratch", (BC, N), f16, kind="Internal").ap()
    cm_dram = nc.dram_tensor("cm_scratch", (BC, N), f32, kind="Internal").ap()

    with tile.TileContext(nc) as tc, ExitStack() as ctx:
        consts = ctx.enter_context(tc.tile_pool(name="consts", bufs=1))
        big = ctx.enter_context(tc.tile_pool(name="big", bufs=2))
        small = ctx.enter_context(tc.tile_pool(name="small", bufs=3))
        psum = ctx.enter_context(tc.tile_pool(name="psum", bufs=2, space="PSUM"))

        # ---- constants ----
        identity = consts.tile([128, 128], f32)
        masks.make_identity(nc, identity)
        identity16 = consts.tile([128, 128], f16)
        nc.vector.tensor_copy(out=identity16, in_=identity)
        ones_row = consts.tile([1, 128], f32)
        nc.vector.memset(ones_row, 1.0)
        ones16 = consts.tile([1, 128], f16)
        nc.vector.memset(ones16, 1.0)
        params_sb = consts.tile([128, 4], f32)
        nc.sync.dma_start(out=params_sb, in_=params.to_broadcast((128, 4)))
        w1t_sb = consts.tile([128, K, H], f32)
        nc.sync.dma_start(out=w1t_sb, in_=w1t.rearrange("(k p) h -> p k h", p=128))
        w2t_sb = consts.tile([H, C], f32)
        nc.sync.dma_start(out=w2t_sb, in_=w2t)

        # ---- persistent stats ----
        mx_cols = consts.tile([128, BC, K, NJ], f32)
        sum_cols = consts.tile([128, BC, K, NJ], f32)
        stats = consts.tile([128, K, BC, 2], f32)   # per (k, b): [sum, max]
        gate_sb = consts.tile([128, K, BC], f32)
        # stationary free >= 2: duplicate each sample's gate into a column
        # pair so both output rows carry the same sample.
        gate_dup = consts.tile([128, K, BC, 2], f16)
        cx_t = consts.tile([128, BC, NB], f32)
        cmrows = consts.tile([NB, BC, 128], f32)

        xrs = [x[b].rearrange("(k p) n -> p k n", p=128) for b in range(BC)]
        x16rs = [x16s[b].rearrange("(k p) n -> p k n", p=128) for b in range(BC)]
        outrs = [out[b].rearrange("(k p) n -> p k n", p=128) for b in range(BC)]

        # ---------------- pass 1: per-channel sum & max over n -------------
        # ScalarE: one activation per tile computes the f32 accum (channel
        # sum) AND the f16 downcast; VectorE takes the max off the f16 copy
        # (16-bit 2x rate); the f16 tile streams back to DRAM.
        def p1_iter(b, j):
            for k in range(K):
                xin = big.tile([128, NT], f32, tag="xin", bufs=3, name="xin")
                nc.sync.dma_start(out=xin, in_=xrs[b][:, k, j * NT:(j + 1) * NT])
                x16t = big.tile([128, NT], f16, tag="x16", bufs=6, name="x16t")
                nc.scalar.activation(
                    out=x16t, in_=xin, func=AF.Copy,
                    accum_out=sum_cols[:, b, k, j:j + 1])
                nc.vector.reduce_max(out=mx_cols[:, b, k, j:j + 1],
                                     in_=x16t, axis=X)
                nc.sync.dma_start(out=x16rs[b][:, k, j * NT:(j + 1) * NT],
                                  in_=x16t)

        # ---------------- MLP -> gate (per sample) -------------------------
        def mlp(b):
            nc.vector.reduce_sum(out=stats[:, :, b, 0:1],
                                 in_=sum_cols[:, b, :, :], axis=X)
            nc.vector.reduce_max(out=stats[:, :, b, 1:2],
                                 in_=mx_cols[:, b, :, :], axis=X)
            h_psum = psum.tile([H, 2], f32, tag="tp", name="h_psum")
            for k in range(K):
                nc.tensor.matmul(h_psum, lhsT=w1t_sb[:, k, :],
                                 rhs=stats[:, k, b, :],
                                 start=(k == 0), stop=(k == K - 1))
            hr = small.tile([H, 2], f32, tag="hr")
            nc.scalar.activation(out=hr[:, 0:1], in_=h_psum[:, 0:1],
                                 func=AF.Relu, scale=1.0 / N)
            nc.scalar.activation(out=hr[:, 1:2], in_=h_psum[:, 1:2],
                                 func=AF.Relu, scale=1.0)
            hsum = small.tile([H, 1], f32, tag="hsum")
            nc.vector.tensor_add(out=hsum, in0=hr[:, 0:1], in1=hr[:, 1:2])
            for k in range(K):
                g_psum = psum.tile([128, 1], f32, tag="tp", name="g_psum")
                nc.tensor.matmul(g_psum, lhsT=w2t_sb[:, k * 128:(k + 1) * 128],
                                 rhs=hsum, start=True, stop=True)
                nc.scalar.activation(out=gate_sb[:, k, b:b + 1], in_=g_psum,
                                     func=AF.Sigmoid)
                for i2 in range(2):
                    nc.scalar.activation(
                        out=gate_dup[:, k, b, i2:i2 + 1],
                        in_=g_psum, func=AF.Sigmoid)

        # ---------------- pass 2: x1 stats over c --------------------------
        # Engine split (measured rates: ACT scale 1.2ns/el; DVE scale
        # 0.43ns/el, TT max f16 0.66ns/el, reduce 1.04ns/el): ScalarE
        # scales 5 chunks, VectorE scales 3 and folds the 7-way running
        # max; one packed PSUM reduce per tile finishes the c-max.
        NSC = 4  # ScalarE scales chunks 0..NSC (k=0 init + 1..NSC), rest DVE

        def p2_iter(b, j):
            # c-sum: gate pair (stationary, f16) @ x16 rows -> [2, 512]
            # row-pieces accumulated across k in a 4-bank PSUM tile.
            cm_bank = psum.tile([2, MV2, MVW], f32, tag="cmb", bufs=1,
                                name="cm_bank")
            tmax = big.tile([128, NT2], f16, tag="tmax")
            for k in range(K):
                x16t = big.tile([128, NT2], f16, tag="x2in", bufs=5,
                                name="x2in")
                nc.sync.dma_start(out=x16t,
                                  in_=x16rs[b][:, k, j * NT2:(j + 1) * NT2])
                for p8 in range(MV2):
                    nc.tensor.matmul(
                        cm_bank[:, p8, :],
                        lhsT=gate_dup[:, k, b, :],
                        rhs=x16t[:, p8 * MVW:(p8 + 1) * MVW],
                        start=(k == 0), stop=(k == K - 1))
                # x1 = gate * x16; running max over chunks on VectorE
                if k == 0:
                    nc.scalar.activation(out=tmax, in_=x16t, func=AF.Copy,
                                         scale=gate_sb[:, k, b:b + 1])
                else:
                    x1 = big.tile([128, NT2], f16,
                                  tag="x1s" if k <= NSC else "x1v",
                                  name="x1")
                    if k <= NSC:
                        nc.scalar.activation(out=x1, in_=x16t, func=AF.Copy,
                                             scale=gate_sb[:, k, b:b + 1])
                    else:
                        nc.vector.tensor_scalar(
                            out=x1, in0=x16t,
                            scalar1=gate_sb[:, k, b:b + 1], scalar2=None,
                            op0=ALU.mult)
                    nc.vector.tensor_tensor(out=tmax, in0=x1, in1=tmax,
                                            op=ALU.max)
            # cm: one ScalarE copy of row 0, one DMA to DRAM
            cm_stage = small.tile([1, MV2, MVW], f32, tag="cmstage",
                                  name="cm_stage")
            nc.scalar.copy(out=cm_stage, in_=cm_bank[0:1, :, :])
            nc.sync.dma_start(
                out=cm_dram[b:b + 1, j * NT2:(j + 1) * NT2].rearrange(
                    "a (p w) -> a p w", w=MVW),
                in_=cm_stage)
            # max over c: transpose all 16 blocks into one packed PSUM
            # tile (f16, 2 banks), one reduce
            tp = psum.tile([128, BPT2, 128], f16, tag="tp")
            for q in range(BPT2):
                nc.tensor.transpose(tp[:, q, :],
                                    tmax[:, q * 128:(q + 1) * 128],
                                    identity16)
            col = j * BPT2
            nc.vector.reduce_max(out=cx_t[:, b, col:col + BPT2], in_=tp,
                                 axis=X)

        # ---------------- softmax over n (transpose layout) ----------------
        def softmax(b):
            nc.sync.dma_start(
                out=cmrows[:, b, :],
                in_=cm_dram[b].rearrange("(jj p) -> jj p", p=128))
            cmt_psum = psum.tile([128, NB], f32, tag="tp", name="cmt_psum")
            nc.tensor.transpose(cmt_psum, cmrows[:, b, :],
                                identity[0:NB, 0:NB])
            s_t = small.tile([128, NB], f32, tag="st")
            # s = sw0 * cx + (sw1/C) * cm_sum
            nc.vector.tensor_scalar(out=s_t, in0=cmt_psum,
                                    scalar1=params_sb[:, 1:2], scalar2=None,
                                    op0=ALU.mult)
            tmp_t = small.tile([128, NB], f32, tag="st2")
            nc.vector.tensor_scalar(out=tmp_t, in0=cx_t[:, b, :],
                                    scalar1=params_sb[:, 0:1], scalar2=None,
                                    op0=ALU.mult)
            nc.vector.tensor_add(out=s_t, in0=s_t, in1=tmp_t)
            # BN (affine, host-folded) + relu
            nc.scalar.activation(out=s_t, in_=s_t, func=AF.Relu,
                                 scale=params_sb[:, 2:3],
                                 bias=params_sb[:, 3:4])
            # global sum over all partitions via PE transpose + ones
            # broadcast.  No max subtraction: s >= 0 (relu) and bounded
            # (~12), so exp(s) stays far inside f32 range.
            def preduce(col, op, nm):
                row_ps = psum.tile([1, 128], f32, tag="tp", name=nm + "_r")
                nc.tensor.transpose(row_ps, col, identity)
                scl = small.tile([1, 1], f32, tag=nm + "s", name=nm + "_s")
                nc.vector.tensor_reduce(out=scl, in_=row_ps, axis=X, op=op)
                rep_ps = psum.tile([128, 1], f32, tag="tp", name=nm + "_b")
                nc.tensor.matmul(rep_ps, lhsT=ones_row, rhs=scl,
                                 start=True, stop=True)
                rep = small.tile([128, 1], f32, tag=nm, name=nm)
                nc.scalar.copy(out=rep, in_=rep_ps)
                return rep
            e_t = small.tile([128, NB], f32, tag="et")
            sume = small.tile([128, 1], f32, tag="sume")
            nc.scalar.activation(out=e_t, in_=s_t, func=AF.Exp,
                                 scale=1.0, accum_out=sume)
            gsum = preduce(sume, ALU.add, "gsum")
            rinv = small.tile([128, 1], f32, tag="rinv")
            nc.vector.reciprocal(out=rinv, in_=gsum)
            att_t = small.tile([128, NB], f32, tag="attt")
            nc.vector.tensor_scalar(out=att_t, in0=e_t, scalar1=rinv,
                                    scalar2=None, op0=ALU.mult)
            # transpose-layout -> row-major (jj on partitions), then store
            attt_psum = psum.tile([NB, 128], f32, tag="tp", name="attt_psum")
            nc.tensor.transpose(attt_psum, att_t, identity)
            att_rows = small.tile([NB, 128], f16, tag="attrows")
            nc.scalar.copy(out=att_rows, in_=attt_psum)
            nc.sync.dma_start(
                out=att_dram[b].rearrange("(jj p) -> jj p", p=128),
                in_=att_rows)

        # ---------------- pass 3: out = att * gate * x16 -------------------
        # Split into loads/compute so the emission schedule can queue the
        # big x16 loads BEFORE any softmax-dependent DMA: a dma_start
        # whose semaphore isn't ready head-of-line-blocks every later
        # dma_start on the sync sequencer (the phase-boundary DMA dips).
        def p3_loads(b, j):
            x16ts = []
            for k in range(K):
                x16t = big.tile([128, NT], f16, tag="x16", bufs=6,
                                name="x16t")
                nc.sync.dma_start(out=x16t,
                                  in_=x16rs[b][:, k, j * NT:(j + 1) * NT])
                x16ts.append(x16t)
            return x16ts

        def p3_compute(b, j, x16ts):
            att_piece = small.tile([1, NT], f16, tag="attp", bufs=1)
            nc.sync.dma_start(out=att_piece,
                              in_=att_dram[b:b + 1, j * NT:(j + 1) * NT])
            attr = big.tile([128, NT], f16, tag="attr")
            for p8 in range(NT // 512):
                bc_psum = psum.tile([128, 512], f32, tag="tp", name="bc_psum")
                nc.tensor.matmul(bc_psum, lhsT=ones16,
                                 rhs=att_piece[:, p8 * 512:(p8 + 1) * 512],
                                 start=True, stop=True)
                nc.scalar.copy(out=attr[:, p8 * 512:(p8 + 1) * 512],
                               in_=bc_psum)
            for k in range(K):
                yout = big.tile([128, NT], f32, tag="xin", bufs=3,
                                name="yout")
                nc.vector.scalar_tensor_tensor(
                    out=yout, in0=x16ts[k], scalar=gate_sb[:, k, b:b + 1],
                    in1=attr, op0=ALU.mult, op1=ALU.mult)
                nc.sync.dma_start(out=outrs[b][:, k, j * NT:(j + 1) * NT],
                                  in_=yout)

        def p3_iter(b, j):
            p3_compute(b, j, p3_loads(b, j))

        # ---------------- emission schedule --------------------------------
        # Cross-sample software pipeline: P2's engine-heavy tiles ride
        # under the DMA-heavy P1/P3 streams of the other sample, so every
        # phase is DMA-bound.
        if BC == 2 and NJ2 % NJ == 0:
            r = NJ2 // NJ
            for j in range(NJ):                      # A: P1(b0)
                p1_iter(0, j)
            mlp(0)
            for j in range(NJ):                      # B: P1(b1) + P2(b0)
                p1_iter(1, j)
                if j == NJ - 1:
                    mlp(1)       # overlap MLP(b1) with the last P2(b0) pair
                for q in range(r):
                    p2_iter(0, j * r + q)
            softmax(0)
            for j in range(NJ):                      # C: P3(b0) + P2(b1)
                p3_iter(0, j)
                for q in range(r):
                    p2_iter(1, j * r + q)
            softmax(1)
            for j in range(NJ):                      # D: P3(b1)
                p3_iter(1, j)
        else:
            for b in range(BC):
                for j in range(NJ):
                    p1_iter(b, j)
                mlp(b)
            for b in range(BC):
                for j in range(NJ2):
                    p2_iter(b, j)
                softmax(b)
            for b in range(BC):
                for j in range(NJ):
                    p3_iter(b, j)

    nc.compile()
    return nc


def _get_nc(NT=4096):
    key = ("nc", NT)
    if key not in _cached:
        _cached[key] = _build_nc(NT)
    return _cached[key]


def _host_params(sw, gamma, beta, running_mean, running_var):
    A = float(gamma[0]) / np.sqrt(float(running_var[0]) + BN_EPS)
    Bconst = float(beta[0]) - float(running_mean[0]) * A
    return np.array([[float(sw[0]), float(sw[1]) / C, A, Bconst]],
                    dtype=np.float32)


def _make_in_maps(x, w1, w2, sw, gamma, beta, running_mean, running_var):
    x = np.ascontiguousarray(np.asarray(x, dtype=np.float32))
    w1t = np.ascontiguousarray(np.asarray(w1, dtype=np.float32).T)
    w2t = np.ascontiguousarray(np.asarray(w2, dtype=np.float32).T)
    params = _host_params(np.asarray(sw), np.asarray(gamma), np.asarray(beta),
                          np.asarray(running_mean), np.asarray(running_var))
    in_maps = []
    for core in range(N_CORES):
        xs = np.ascontiguousarray(x[core * BC:(core + 1) * BC])
        in_maps.append({"x": xs, "w1t": w1t, "w2t": w2t, "params": params})
    return in_maps


def run_sharded(inputs, trace=False, NT=4096):
    """Run on all 8 cores; returns (out_full, BassKernelResults)."""
    from concourse.bass_utils import run_bass_kernel_spmd

    nc = _get_nc(NT)
    in_maps = _make_in_maps(**inputs)
    res = run_bass_kernel_spmd(nc, in_maps, core_ids=list(range(N_CORES)),
                               trace=trace)
    out = np.concatenate([r["out"] for r in res.results], axis=0)
    return out, res


def kernel(**inputs) -> np.ndarray:
    out, _ = run_sharded(inputs, trace=False)
    return out


# revision 40
# speedup vs baseline: 1.0274x; 1.0274x over previous
"""CBAM attention module (channel gate + spatial softmax attention) on 8 TRN2
NeuronCores, data-parallel over the batch dimension.

Reference computation (per sample b):
    m  = mean_n x[c, n];  mx = max_n x[c, n]
    gate = sigmoid(w2 @ (relu(w1 @ m) + relu(w1 @ mx)))          # (C,)
    x1 = gate[:, None] * x
    s  = sw0 * max_c x1 + sw1 * mean_c x1                        # (N,)
    s  = relu(A * s + Bconst)        # BatchNorm1d(1) eval, folded on host
    att = softmax_n(s)
    out = att[None, :] * x1

Kernel structure per core (2 samples each), HBM-traffic-minimized:
    pass 1: stream x (f32), per-channel sum over n (ScalarE activation
            accum, which also downcasts the tile to f16) and max over n
            (VectorE reduce on the f16 copy); the f16 copy is written back
            to DRAM.  Tiny MLP on TensorE -> gate.
    pass 2: stream x16 (f16); gate-scale split ScalarE(5 chunks)/
            VectorE(3 chunks, 4x tensor_scalar), running-max fold on
            VectorE (f16 TT, 2x); TensorE transposes the 16 blocks per
            tile into one packed f16 PSUM tile, one VectorE reduce
            finishes max-over-c; TensorE matvec (gate stationary, f16)
            accumulates the c-sum in PSUM row-pieces.  Softmax over n in
            the transposed layout (no max-subtract: s>=0 and bounded);
            att replicated across partitions on TensorE.
    pass 3: stream x16 again; out = (x16 * gate) * att on VectorE
            (scalar_tensor_tensor), write back f32.

HBM bytes per core: 134R(f32) + 67W(f16) + 67R + 67R + 134W(f32 out)
= 469 MB vs 536 MB for the all-f32 3R1W scheme.  DMA-bound at
~358 GB/s -> ~1.31 ms floor; measured 1.33-1.41 ms (device-state
noise ~5%).  f16 staging costs ~2.6e-4 relative error, far inside the
2e-2 gate.  The cross-sample emission pipeline (P1(b1)+P2(b0), then
P3(b0)+P2(b1)) keeps every phase DMA-bound; pass-3 emits its eight
x16 loads before the att-piece DMA so the softmax semaphore wait
cannot head-of-line-block the sync sequencer's load stream.
"""

import numpy as np

B, C, N, RATIO = 16, 1024, 16384, 8
H = C // RATIO  # 128
BN_EPS = 1e-5
N_CORES = 8
BC = B // N_CORES  # samples per core

_cached = {}


def _build_nc(NT=4096, BC=BC, C=C, N=N, H=H):
    import concourse.bacc as bacc
    import concourse.mybir as mybir
    import concourse.tile as tile
    from concourse import masks
    from contextlib import ExitStack

    f32 = mybir.dt.float32
    f16 = mybir.dt.float16
    AF = mybir.ActivationFunctionType
    X = mybir.AxisListType.X
    ALU = mybir.AluOpType

    K = C // 128          # c-chunks
    NJ = N // NT          # n-tiles per sample (passes 1/3)
    NB = N // 128         # 128-blocks per sample (transpose-layout columns)
    NT2 = min(2048, NT)   # pass-2 tile (PSUM banks cap the matvec pieces)
    NJ2 = N // NT2
    BPT2 = NT2 // 128     # 128-blocks per pass-2 tile
    MV2 = max(1, NT2 // 512)  # matvec row-pieces per pass-2 tile
    MVW = min(NT2, 512)
    assert NB <= 128

    nc = bacc.Bacc("TRN2", target_bir_lowering=False, debug=False,
                   num_devices=N_CORES)

    x = nc.dram_tensor("x", (BC, C, N), f32, kind="ExternalInput").ap()
    w1t = nc.dram_tensor("w1t", (C, H), f32, kind="ExternalInput").ap()
    w2t = nc.dram_tensor("w2t", (H, C), f32, kind="ExternalInput").ap()
    # params = [sw0, sw1/C, A, Bconst]
    params = nc.dram_tensor("params", (1, 4), f32, kind="ExternalInput").ap()
    out = nc.dram_tensor("out", (BC, C, N), f32, kind="ExternalOutput").ap()

    # f16 staging copy of x, written in pass 1, read in passes 2 and 3.
    # One tensor per sample so pass-2 reads of sample b only wait on the
    # pass-1 writes of the same sample.
    x16s = [nc.dram_tensor(f"x16_{b}", (C, N), f16, kind="Internal").ap()
            for b in range(BC)]
    att_dram = nc.dram_tensor("att_scratch", (BC, N), f16, kind="Internal").ap()
    cm_dram = nc.dram_tensor("cm_scratch", (BC, N), f32, kind="Internal").ap()

    with tile.TileContext(nc) as tc, ExitStack() as ctx:
        consts = ctx.enter_context(tc.tile_pool(name="consts", bufs=1))
        big = ctx.enter_context(tc.tile_pool(name="big", bufs=2))
        small = ctx.enter_context(tc.tile_pool(name="small", bufs=3))
        psum = ctx.enter_context(tc.tile_pool(name="psum", bufs=2, space="PSUM"))

        # ---- constants ----
        identity = consts.tile([128, 128], f32)
        masks.make_identity(nc, identity)
        identity16 = consts.tile([128, 128], f16)
        nc.vector.tensor_copy(out=identity16, in_=identity)
        ones_row = consts.tile([1, 128], f32)
        nc.vector.memset(ones_row, 1.0)
        ones16 = consts.tile([1, 128], f16)
        nc.vector.memset(ones16, 1.0)
        params_sb = consts.tile([128, 4], f32)
        nc.sync.dma_start(out=params_sb, in_=params.to_broadcast((128, 4)))
        w1t_sb = consts.tile([128, K, H], f32)
        nc.sync.dma_start(out=w1t_sb, in_=w1t.rearrange("(k p) h -> p k h", p=128))
        w2t_sb = consts.tile([H, C], f32)
        nc.sync.dma_start(out=w2t_sb, in_=w2t)

        # ---- persistent stats ----
        mx_cols = consts.tile([128, BC, K, NJ], f32)
        sum_cols = consts.tile([128, BC, K, NJ], f32)
        stats = consts.tile([128, K, BC, 2], f32)   # per (k, b): [sum, max]
        gate_sb = consts.tile([128, K, BC], f32)
        # stationary free >= 2: duplicate each sample's gate into a column
        # pair so both output rows carry the same sample.
        gate_dup = consts.tile([128, K, BC, 2], f16)
        cx_t = consts.tile([128, BC, NB], f32)
        cmrows = consts.tile([NB, BC, 128], f32)

        xrs = [x[b].rearrange("(k p) n -> p k n", p=128) for b in range(BC)]
        x16rs = [x16s[b].rearrange("(k p) n -> p k n", p=128) for b in range(BC)]
        outrs = [out[b].rearrange("(k p) n -> p k n", p=128) for b in range(BC)]

        # ---------------- pass 1: per-channel sum & max over n -------------
        # ScalarE: one activation per tile computes the f32 accum (channel
        # sum) AND the f16 downcast; VectorE takes the max off the f16 copy
        # (16-bit 2x rate); the f16 tile streams back to DRAM.
        def p1_iter(b, j):
            for k in range(K):
                xin = big.tile([128, NT], f32, tag="xin", bufs=3, name="xin")
                nc.sync.dma_start(out=xin, in_=xrs[b][:, k, j * NT:(j + 1) * NT])
                x16t = big.tile([128, NT], f16, tag="x16", bufs=6, name="x16t")
                nc.scalar.activation(
                    out=x16t, in_=xin, func=AF.Copy,
                    accum_out=sum_cols[:, b, k, j:j + 1])
                nc.vector.reduce_max(out=mx_cols[:, b, k, j:j + 1],
                                     in_=x16t, axis=X)
                nc.sync.dma_start(out=x16rs[b][:, k, j * NT:(j + 1) * NT],
                                  in_=x16t)

        # ---------------- MLP -> gate (per sample) -------------------------
        def mlp(b):
            nc.vector.reduce_sum(out=stats[:, :, b, 0:1],
                                 in_=sum_cols[:, b, :, :], axis=X)
            nc.vector.reduce_max(out=stats[:, :, b, 1:2],
                                 in_=mx_cols[:, b, :, :], axis=X)
            h_psum = psum.tile([H, 2], f32, tag="tp", name="h_psum")
            for k in range(K):
                nc.tensor.matmul(h_psum, lhsT=w1t_sb[:, k, :],
                                 rhs=stats[:, k, b, :],
                                 start=(k == 0), stop=(k == K - 1))
            hr = small.tile([H, 2], f32, tag="hr")
            nc.scalar.activation(out=hr[:, 0:1], in_=h_psum[:, 0:1],
                                 func=AF.Relu, scale=1.0 / N)
            nc.scalar.activation(out=hr[:, 1:2], in_=h_psum[:, 1:2],
                                 func=AF.Relu, scale=1.0)
            hsum = small.tile([H, 1], f32, tag="hsum")
            nc.vector.tensor_add(out=hsum, in0=hr[:, 0:1], in1=hr[:, 1:2])
            for k in range(K):
                g_psum = psum.tile([128, 1], f32, tag="tp", name="g_psum")
                nc.tensor.matmul(g_psum, lhsT=w2t_sb[:, k * 128:(k + 1) * 128],
                                 rhs=hsum, start=True, stop=True)
                nc.scalar.activation(out=gate_sb[:, k, b:b + 1], in_=g_psum,
                                     func=AF.Sigmoid)
                for i2 in range(2):
                    nc.scalar.activation(
                        out=gate_dup[:, k, b, i2:i2 + 1],
                        in_=g_psum, func=AF.Sigmoid)

        # ---------------- pass 2: x1 stats over c --------------------------
        # Engine split (measured rates: ACT scale 1.2ns/el; DVE scale
        # 0.43ns/el, TT max f16 0.66ns/el, reduce 1.04ns/el): ScalarE
        # scales 5 chunks, VectorE scales 3 and folds the 7-way running
        # max; one packed PSUM reduce per tile finishes the c-max.
        NSC = 4  # ScalarE scales chunks 0..NSC (k=0 init + 1..NSC), rest DVE

        def p2_iter(b, j):
            # c-sum: gate pair (stationary, f16) @ x16 rows -> [2, 512]
            # row-pieces accumulated across k in a 4-bank PSUM tile.
            cm_bank = psum.tile([2, MV2, MVW], f32, tag="cmb", bufs=1,
                                name="cm_bank")
            tmax = big.tile([128, NT2], f16, tag="tmax")
            for k in range(K):
                x16t = big.tile([128, NT2], f16, tag="x2in", bufs=5,
                                name="x2in")
                nc.sync.dma_start(out=x16t,
                                  in_=x16rs[b][:, k, j * NT2:(j + 1) * NT2])
                for p8 in range(MV2):
                    nc.tensor.matmul(
                        cm_bank[:, p8, :],
                        lhsT=gate_dup[:, k, b, :],
                        rhs=x16t[:, p8 * MVW:(p8 + 1) * MVW],
                        start=(k == 0), stop=(k == K - 1))
                # x1 = gate * x16; running max over chunks on VectorE
                if k == 0:
                    nc.scalar.activation(out=tmax, in_=x16t, func=AF.Copy,
                                         scale=gate_sb[:, k, b:b + 1])
                else:
                    x1 = big.tile([128, NT2], f16,
                                  tag="x1s" if k <= NSC else "x1v",
                                  name="x1")
                    if k <= NSC:
                        nc.scalar.activation(out=x1, in_=x16t, func=AF.Copy,
                                             scale=gate_sb[:, k, b:b + 1])
                    else:
                        nc.vector.tensor_scalar(
                            out=x1, in0=x16t,
                            scalar1=gate_sb[:, k, b:b + 1], scalar2=None,
                            op0=ALU.mult)
                    nc.vector.tensor_tensor(out=tmax, in0=x1, in1=tmax,
                                            op=ALU.max)
            # cm: one ScalarE copy of row 0, one DMA to DRAM
            cm_stage = small.tile([1, MV2, MVW], f32, tag="cmstage",
                                  name="cm_stage")
            nc.scalar.copy(out=cm_stage, in_=cm_bank[0:1, :, :])
            nc.sync.dma_start(
                out=cm_dram[b:b + 1, j * NT2:(j + 1) * NT2].rearrange(
                    "a (p w) -> a p w", w=MVW),
                in_=cm_stage)
            # max over c: transpose all 16 blocks into one packed PSUM
            # tile (f16, 2 banks), one reduce
            tp = psum.tile([128, BPT2, 128], f16, tag="tp")
            for q in range(BPT2):
                nc.tensor.transpose(tp[:, q, :],
                                    tmax[:, q * 128:(q + 1) * 128],
                                    identity16)
            col = j * BPT2
            nc.vector.reduce_max(out=cx_t[:, b, col:col + BPT2], in_=tp,
                                 axis=X)

        # ---------------- softmax over n (transpose layout) ----------------
        def softmax(b):
            nc.sync.dma_start(
                out=cmrows[:, b, :],
                in_=cm_dram[b].rearrange("(jj p) -> jj p", p=128))
            cmt_psum = psum.tile([128, NB], f32, tag="tp", name="cmt_psum")
            nc.tensor.transpose(cmt_psum, cmrows[:, b, :],
                                identity[0:NB, 0:NB])
            s_t = small.tile([128, NB], f32, tag="st")
            # s = sw0 * cx + (sw1/C) * cm_sum
            nc.vector.tensor_scalar(out=s_t, in0=cmt_psum,
                                    scalar1=params_sb[:, 1:2], scalar2=None,
                                    op0=ALU.mult)
            tmp_t = small.tile([128, NB], f32, tag="st2")
            nc.vector.tensor_scalar(out=tmp_t, in0=cx_t[:, b, :],
                                    scalar1=params_sb[:, 0:1], scalar2=None,
                                    op0=ALU.mult)
            nc.vector.tensor_add(out=s_t, in0=s_t, in1=tmp_t)
            # BN (affine, host-folded) + relu
            nc.scalar.activation(out=s_t, in_=s_t, func=AF.Relu,
                                 scale=params_sb[:, 2:3],
                                 bias=params_sb[:, 3:4])
            # global sum over all partitions via PE transpose + ones
            # broadcast.  No max subtraction: s >= 0 (relu) and bounded
            # (~12), so exp(s) stays far inside f32 range.
            def preduce(col, op, nm):
                row_ps = psum.tile([1, 128], f32, tag="tp", name=nm + "_r")
                nc.tensor.transpose(row_ps, col, identity)
                scl = small.tile([1, 1], f32, tag=nm + "s", name=nm + "_s")
                nc.vector.tensor_reduce(out=scl, in_=row_ps, axis=X, op=op)
                rep_ps = psum.tile([128, 1], f32, tag="tp", name=nm + "_b")
                nc.tensor.matmul(rep_ps, lhsT=ones_row, rhs=scl,
                                 start=True, stop=True)
                rep = small.tile([128, 1], f32, tag=nm, name=nm)
                nc.scalar.copy(out=rep, in_=rep_ps)
                return rep
            e_t = small.tile([128, NB], f32, tag="et")
            sume = small.tile([128, 1], f32, tag="sume")
            nc.scalar.activation(out=e_t, in_=s_t, func=AF.Exp,
                                 scale=1.0, accum_out=sume)
            gsum = preduce(sume, ALU.add, "gsum")
            rinv = small.tile([128, 1], f32, tag="rinv")
            nc.vector.reciprocal(out=rinv, in_=gsum)
            att_t = small.tile([128, NB], f32, tag="attt")
            nc.vector.tensor_scalar(out=att_t, in0=e_t, scalar1=rinv,
                                    scalar2=None, op0=ALU.mult)
            # transpose-layout -> row-major (jj on partitions), then store
            attt_psum = psum.tile([NB, 128], f32, tag="tp", name="attt_psum")
            nc.tensor.transpose(attt_psum, att_t, identity)
            att_rows = small.tile([NB, 128], f16, tag="attrows")
            nc.scalar.copy(out=att_rows, in_=attt_psum)
            nc.sync.dma_start(
                out=att_dram[b].rearrange("(jj p) -> jj p", p=128),
                in_=att_rows)

        # ---------------- pass 3: out = att * gate * x16 -------------------
        # Split into loads/compute so the emission schedule can queue the
        # big x16 loads BEFORE any softmax-dependent DMA: a dma_start
        # whose semaphore isn't ready head-of-line-blocks every later
        # dma_start on the sync sequencer (the phase-boundary DMA dips).
        def p3_loads(b, j):
            x16ts = []
            for k in range(K):
                x16t = big.tile([128, NT], f16, tag="x16", bufs=6,
                                name="x16t")
                nc.sync.dma_start(out=x16t,
                                  in_=x16rs[b][:, k, j * NT:(j + 1) * NT])
                x16ts.append(x16t)
            return x16ts

        def p3_compute(b, j, x16ts):
            att_piece = small.tile([1, NT], f16, tag="attp", bufs=1)
            nc.sync.dma_start(out=att_piece,
                              in_=att_dram[b:b + 1, j * NT:(j + 1) * NT])
            attr = big.tile([128, NT], f16, tag="attr")
            for p8 in range(NT // 512):
                bc_psum = psum.tile([128, 512], f32, tag="tp", name="bc_psum")
                nc.tensor.matmul(bc_psum, lhsT=ones16,
                                 rhs=att_piece[:, p8 * 512:(p8 + 1) * 512],
                                 start=True, stop=True)
                nc.scalar.copy(out=attr[:, p8 * 512:(p8 + 1) * 512],
                               in_=bc_psum)
            for k in range(K):
                yout = big.tile([128, NT], f32, tag="xin", bufs=3,
                                name="yout")
                nc.vector.scalar_tensor_tensor(
                    out=yout, in0=x16ts[k], scalar=gate_sb[:, k, b:b + 1],
                    in1=attr, op0=ALU.mult, op1=ALU.mult)
                nc.sync.dma_start(out=outrs[b][:, k, j * NT:(j + 1) * NT],
                                  in_=yout)

        def p3_iter(b, j):
            p3_compute(b, j, p3_loads(b, j))

        # ---------------- emission schedule --------------------------------
        # Cross-sample software pipeline: P2's engine-heavy tiles ride
        # under the DMA-heavy P1/P3 streams of the other sample, so every
        # phase is DMA-bound.
        if BC == 2 and NJ2 % NJ == 0:
            r = NJ2 // NJ
            for j in range(NJ):                      # A: P1(b0)
                p1_iter(0, j)
            mlp(0)
            for j in range(NJ):                      # B: P1(b1) + P2(b0)
                p1_iter(1, j)
                if j == NJ - 1:
                    mlp(1)       # overlap MLP(b1) with the last P2(b0) pair
                for q in range(r):
                    p2_iter(0, j * r + q)
            softmax(0)
            for j in range(NJ):                      # C: P3(b0) + P2(b1)
                p3_iter(0, j)
                for q in range(r):
                    p2_iter(1, j * r + q)
            softmax(1)
            for j in range(NJ):                      # D: P3(b1)
                p3_iter(1, j)
        else:
            for b in range(BC):
                for j in range(NJ):
                    p1_iter(b, j)
                mlp(b)
            for b in range(BC):
                for j in range(NJ2):
                    p2_iter(b, j)
                softmax(b)
            for b in range(BC):
                for j in range(NJ):
                    p3_iter(b, j)

    nc.compile()
    return nc


def _get_nc(NT=4096):
    key = ("nc", NT)
    if key not in _cached:
        _cached[key] = _build_nc(NT)
    return _cached[key]


def _host_params(sw, gamma, beta, running_mean, running_var):
    A = float(gamma[0]) / np.sqrt(float(running_var[0]) + BN_EPS)
    Bconst = float(beta[0]) - float(running_mean[0]) * A
    return np.array([[float(sw[0]), float(sw[1]) / C, A, Bconst]],
                    dtype=np.float32)


def _make_in_maps(x, w1, w2, sw, gamma, beta, running_mean, running_var):
    x = np.ascontiguousarray(np.asarray(x, dtype=np.float32))
    w1t = np.ascontiguousarray(np.asarray(w1, dtype=np.float32).T)
    w2t = np.ascontiguousarray(np.asarray(w2, dtype=np.float32).T)
    params = _host_params(np.asarray(sw), np.asarray(gamma), np.asarray(beta),
                          np.asarray(running_mean), np.asarray(running_var))
    in_maps = []
    for core in range(N_CORES):
        xs = np.ascontiguousarray(x[core * BC:(core + 1) * BC])
        in_maps.append({"x": xs, "w1t": w1t, "w2t": w2t, "params": params})
    return in_maps


def run_sharded(inputs, trace=False, NT=4096):
    """Run on all 8 cores; returns (out_full, BassKernelResults)."""
    from concourse.bass_utils import run_bass_kernel_spmd

    nc = _get_nc(NT)
    in_maps = _make_in_maps(**inputs)
    res = run_bass_kernel_spmd(nc, in_maps, core_ids=list(range(N_CORES)),
                               trace=trace)
    out = np.concatenate([r["out"] for r in res.results], axis=0)
    return out, res


def kernel(**inputs) -> np.ndarray:
    out, _ = run_sharded(inputs, trace=False)
    return out
